# revision 5
# baseline (speedup 1.0000x reference)
"""Trainium2 Bass kernel for attention pooling (nn_AtnPool).

Math (per batch b):
  h[s,k']   = gelu( f[s,:] @ W1[:,k'] + b1[k'] )        k' = h*64+k, [2048, 512]
  score     = h @ blockdiag(w2)                          [2048, 1024] (per head o-block)
  w         = softmax_s(score + b2)  == softmax_s(score) (b2 const along s -> drops out)
  out[d]    = sum_s f[s,d] * w[s, d]                     d = h*128+o

Strategy: data-parallel over batch, 4 batches per core, 8 cores, no
collectives.  All matmuls contract over the partition dim, so features are
shipped pre-transposed (host-side) as fT[d, s] in bf16.  Per head h, the
d-chunk h of fT is exactly f[:, h*128:(h+1)*128]^T, which feeds both the
score matmul and the final fused multiply-reduce (DVE tensor_tensor_reduce).
Softmax denominator Z comes free from the Exp activation's accum_out.
"""

import sys

for _p in ("/opt/trn_rl_repo",):
    if _p not in sys.path:
        sys.path.insert(0, _p)

from contextlib import ExitStack

import ml_dtypes
import numpy as np

import concourse.bass as bass
import concourse.tile as tile
from concourse import bacc, mybir
from concourse.bass_utils import run_bass_kernel_spmd

# Problem shapes (hardcoded per harness contract).
B, S, D = 32, 2048, 1024
H, DH = 8, 64
KP = H * DH      # 512
DHO = D // H     # 128
NCORES = 8
BL = B // NCORES  # 4 batches per core

BF16 = mybir.dt.bfloat16
F32 = mybir.dt.float32
AF = mybir.ActivationFunctionType
ALU = mybir.AluOpType


def build_bass(act="gelu"):
    act_fn = {"gelu": AF.Gelu, "tanh": AF.Tanh}[act]
    nc = bacc.Bacc("TRN2", target_bir_lowering=False, debug=False)

    ftp = nc.declare_dram_parameter("ftp", [BL, 8, 128, S], BF16, isOutput=False)
    w1t = nc.declare_dram_parameter("w1t", [128, 8, KP], BF16, isOutput=False)
    b1v = nc.declare_dram_parameter("b1v", [128, 4], F32, isOutput=False)
    w2p = nc.declare_dram_parameter("w2p", [128, H, DHO], BF16, isOutput=False)
    outp = nc.declare_dram_parameter("outp", [128, BL * H], F32, isOutput=True)

    with tile.TileContext(nc) as tc, ExitStack() as ctx:
        singles = ctx.enter_context(tc.tile_pool(name="singles", bufs=1))
        ftpool = ctx.enter_context(tc.tile_pool(name="ft", bufs=16))
        hpool = ctx.enter_context(tc.tile_pool(name="h", bufs=8))
        epool = ctx.enter_context(tc.tile_pool(name="e", bufs=3))
        prodpool = ctx.enter_context(tc.tile_pool(name="prod", bufs=2))
        smalls = ctx.enter_context(tc.tile_pool(name="smalls", bufs=8))
        psum_h = ctx.enter_context(tc.tile_pool(name="psum_h", bufs=2, space="PSUM"))
        psum_e = ctx.enter_context(tc.tile_pool(name="psum_e", bufs=3, space="PSUM"))

        w1s = singles.tile([128, 8, KP], BF16, tag="w1s")
        nc.sync.dma_start(out=w1s, in_=w1t.ap())
        w2s = singles.tile([128, H, DHO], BF16, tag="w2s")
        nc.sync.dma_start(out=w2s, in_=w2p.ap())
        b1s = singles.tile([128, 4], F32, tag="b1s")
        nc.sync.dma_start(out=b1s, in_=b1v.ap())
        outacc = singles.tile([128, BL * H], F32, tag="outacc")

        ftap = ftp.ap()
        for b in range(BL):
            # --- load fT chunks for this batch ---
            ftt = []
            for dc in range(8):
                t = ftpool.tile([128, S], BF16, tag="ft")
                nc.sync.dma_start(out=t, in_=ftap[b, dc])
                ftt.append(t)

            # --- einsum1: h^T[k'-chunk, s] = W1^T fT  (+b1, gelu) ---
            hts = []
            for kc in range(4):
                ht = hpool.tile([128, S], BF16, tag="h")
                for blk in range(4):  # s in blocks of 512
                    ph = psum_h.tile([128, 512], F32, tag="ph")
                    for dc in range(8):
                        nc.tensor.matmul(
                            ph,
                            lhsT=w1s[:, dc, kc * 128:(kc + 1) * 128],
                            rhs=ftt[dc][:, blk * 512:(blk + 1) * 512],
                            start=(dc == 0),
                            stop=(dc == 7),
                        )
                    nc.scalar.activation(
                        out=ht[:, blk * 512:(blk + 1) * 512],
                        in_=ph,
                        func=act_fn,
                        bias=b1s[:, kc:kc + 1],
                    )
                hts.append(ht)

            # --- per head: scores^T[o, s], exp(+Z), fused weighted reduce ---
            for h in range(H):
                kc, slot = h // 2, h % 2
                pb = slot * 64
                eh = epool.tile([128, S], BF16, tag="e")
                zs = smalls.tile([128, 2], F32, tag="zs")
                for half in range(2):  # s in halves of 1024
                    pe_ = psum_e.tile([128, 1024], F32, tag="pe")
                    for g in range(2):
                        nc.tensor.matmul(
                            pe_[:, g * 512:(g + 1) * 512],
                            lhsT=w2s[pb:pb + 64, h, :],
                            rhs=hts[kc][pb:pb + 64,
                                        half * 1024 + g * 512:
                                        half * 1024 + (g + 1) * 512],
                            start=True,
                            stop=True,
                        )
                    nc.scalar.activation(
                        out=eh[:, half * 1024:(half + 1) * 1024],
                        in_=pe_,
                        func=AF.Exp,
                        accum_out=zs[:, half:half + 1],
                    )
                z = smalls.tile([128, 1], F32, tag="z")
                nc.vector.tensor_add(z, zs[:, 0:1], zs[:, 1:2])
                num = smalls.tile([128, 1], F32, tag="num")
                prod = prodpool.tile([128, S], BF16, tag="prod")
                # out = (e * 1.0) * fT ; accum_out = sum_s(out)   (TTR opcode
                # crashes the runtime here, scalar_tensor_tensor works)
                nc.vector.scalar_tensor_tensor(
                    out=prod,
                    in0=eh,
                    in1=ftt[h],
                    scalar=1.0,
                    op0=ALU.mult,
                    op1=ALU.mult,
                    accum_out=num,
                )
                rz = smalls.tile([128, 1], F32, tag="rz")
                nc.vector.reciprocal(rz, z)
                nc.vector.tensor_mul(outacc[:, b * H + h:b * H + h + 1], num, rz)

        nc.sync.dma_start(out=outp.ap(), in_=outacc)

    nc.compile()
    return nc


def prep_inputs(features, w1, b1, w2):
    """Host-side sharding/layout. Returns in_maps for 8 cores."""
    bf = ml_dtypes.bfloat16
    # W1[d, h*64+k] as lhsT chunks: w1t[p, dc, k'] = W1[dc*128+p, k']
    W1 = np.ascontiguousarray(w1.transpose(1, 0, 2).reshape(D, KP))
    w1t = np.ascontiguousarray(
        W1.reshape(8, 128, KP).transpose(1, 0, 2)).astype(bf)
    b1v = np.ascontiguousarray(
        b1.reshape(KP).reshape(4, 128).T).astype(np.float32)
    w2p = np.zeros((128, H, DHO), dtype=bf)
    for h in range(H):
        pb = (h % 2) * 64
        w2p[pb:pb + 64, h, :] = w2[h].astype(bf)

    in_maps = []
    for c in range(NCORES):
        fc = features[c * BL:(c + 1) * BL]  # [BL, S, D]
        ft = np.ascontiguousarray(fc.transpose(0, 2, 1)).astype(bf)
        ftp = ft.reshape(BL, 8, 128, S)
        in_maps.append({"ftp": ftp, "w1t": w1t, "b1v": b1v, "w2p": w2p})
    return in_maps


def assemble_output(results):
    """results: list of 8 dicts with 'outp' [128, BL*H] f32 -> [B, D]."""
    out = np.empty((B, D), dtype=np.float32)
    for c, r in enumerate(results):
        o = np.asarray(r["outp"], dtype=np.float32)  # [128(o), BL*H]
        blk = o.reshape(128, BL, H).transpose(1, 2, 0).reshape(BL, D)
        out[c * BL:(c + 1) * BL] = blk
    return out


_NC_CACHE = {}


def get_nc():
    if "nc" not in _NC_CACHE:
        _NC_CACHE["nc"] = build_bass()
    return _NC_CACHE["nc"]


def kernel(features, mask, lengths, w1, b1, w2, b2, **_ignored):
    # mask is all-ones and lengths unused in the reference forward; b2 is
    # constant along the softmax axis so it cancels in the softmax.
    features = np.asarray(features, dtype=np.float32)
    in_maps = prep_inputs(features, np.asarray(w1, np.float32),
                          np.asarray(b1, np.float32), np.asarray(w2, np.float32))
    nc = get_nc()
    res = run_bass_kernel_spmd(nc, in_maps, core_ids=list(range(NCORES)))
    return assemble_output(res.results)


if __name__ == "__main__":
    rng = np.random.default_rng(0)
    feats = rng.standard_normal((B, S, D), dtype=np.float32)
    w1 = (rng.standard_normal((H, D, DH)) * 0.01).astype(np.float32)
    b1 = (rng.standard_normal((H, DH)) * 0.01).astype(np.float32)
    w2 = (rng.standard_normal((H, DH, DHO)) * 0.01).astype(np.float32)
    b2 = (rng.standard_normal((H, DHO)) * 0.01).astype(np.float32)
    out = kernel(feats, np.ones((B, S), np.int32), None, w1, b1, w2, b2)
    print(out.shape, out.dtype, np.abs(out).mean())


# revision 7
# speedup vs baseline: 6782.8333x; 6782.8333x over previous
"""Trainium2 Bass kernel for attention pooling (nn_AtnPool).

Math (per batch b):
  h[s,k']   = gelu( f[s,:] @ W1[:,k'] + b1[k'] )        k' = h*64+k, [2048, 512]
  score     = h @ blockdiag(w2)                          [2048, 1024] (per head o-block)
  w         = softmax_s(score + b2)  == softmax_s(score) (b2 const along s -> drops out)
  out[d]    = sum_s f[s,d] * w[s, d]                     d = h*128+o

Strategy: data-parallel over batch, 4 batches per core, 8 cores, no
collectives.  All matmuls contract over the partition dim, so features are
shipped pre-transposed (host-side) as fT[d, s] in bf16.  Per head h, the
d-chunk h of fT is exactly f[:, h*128:(h+1)*128]^T, which feeds both the
score matmul and the final fused multiply-reduce (DVE tensor_tensor_reduce).
Softmax denominator Z comes free from the Exp activation's accum_out.
"""

import sys

for _p in ("/opt/trn_rl_repo",):
    if _p not in sys.path:
        sys.path.insert(0, _p)

from contextlib import ExitStack

import ml_dtypes
import numpy as np

import concourse.bass as bass
import concourse.tile as tile
from concourse import bacc, mybir
from concourse.bass_utils import run_bass_kernel_spmd

# Problem shapes (hardcoded per harness contract).
B, S, D = 32, 2048, 1024
H, DH = 8, 64
KP = H * DH      # 512
DHO = D // H     # 128
NCORES = 8
BL = B // NCORES  # 4 batches per core

BF16 = mybir.dt.bfloat16
F32 = mybir.dt.float32
AF = mybir.ActivationFunctionType
ALU = mybir.AluOpType


def build_bass(act="gelu", repeat=1):
    act_fn = {"gelu": AF.Gelu, "tanh": AF.Tanh}[act]
    nc = bacc.Bacc("TRN2", target_bir_lowering=False, debug=False)

    ftp = nc.declare_dram_parameter("ftp", [BL, 8, 128, S], BF16, isOutput=False)
    w1t = nc.declare_dram_parameter("w1t", [128, 8, KP], BF16, isOutput=False)
    b1v = nc.declare_dram_parameter("b1v", [128, 4], F32, isOutput=False)
    w2p = nc.declare_dram_parameter("w2p", [128, H, DHO], BF16, isOutput=False)
    outp = nc.declare_dram_parameter("outp", [128, BL * H], F32, isOutput=True)

    with tile.TileContext(nc) as tc, ExitStack() as ctx:
        singles = ctx.enter_context(tc.tile_pool(name="singles", bufs=1))
        ftpool = ctx.enter_context(tc.tile_pool(name="ft", bufs=16))
        hpool = ctx.enter_context(tc.tile_pool(name="h", bufs=8))
        epool = ctx.enter_context(tc.tile_pool(name="e", bufs=3))
        prodpool = ctx.enter_context(tc.tile_pool(name="prod", bufs=2))
        smalls = ctx.enter_context(tc.tile_pool(name="smalls", bufs=8))
        psum_h = ctx.enter_context(tc.tile_pool(name="psum_h", bufs=2, space="PSUM"))
        psum_e = ctx.enter_context(tc.tile_pool(name="psum_e", bufs=3, space="PSUM"))

        w1s = singles.tile([128, 8, KP], BF16, tag="w1s")
        nc.sync.dma_start(out=w1s, in_=w1t.ap())
        w2s = singles.tile([128, H, DHO], BF16, tag="w2s")
        nc.sync.dma_start(out=w2s, in_=w2p.ap())
        b1s = singles.tile([128, 4], F32, tag="b1s")
        nc.sync.dma_start(out=b1s, in_=b1v.ap())
        outacc = singles.tile([128, BL * H], F32, tag="outacc")

        ftap = ftp.ap()
        for b in [b for _ in range(repeat) for b in range(BL)]:
            # --- load fT chunks for this batch ---
            ftt = []
            for dc in range(8):
                t = ftpool.tile([128, S], BF16, tag="ft")
                nc.sync.dma_start(out=t, in_=ftap[b, dc])
                ftt.append(t)

            # --- einsum1: h^T[k'-chunk, s] = W1^T fT  (+b1, gelu) ---
            hts = []
            for kc in range(4):
                ht = hpool.tile([128, S], BF16, tag="h")
                for blk in range(4):  # s in blocks of 512
                    ph = psum_h.tile([128, 512], F32, tag="ph")
                    for dc in range(8):
                        nc.tensor.matmul(
                            ph,
                            lhsT=w1s[:, dc, kc * 128:(kc + 1) * 128],
                            rhs=ftt[dc][:, blk * 512:(blk + 1) * 512],
                            start=(dc == 0),
                            stop=(dc == 7),
                        )
                    nc.scalar.activation(
                        out=ht[:, blk * 512:(blk + 1) * 512],
                        in_=ph,
                        func=act_fn,
                        bias=b1s[:, kc:kc + 1],
                    )
                hts.append(ht)

            # --- per head: scores^T[o, s], exp(+Z), fused weighted reduce ---
            for h in range(H):
                kc, slot = h // 2, h % 2
                pb = slot * 64
                eh = epool.tile([128, S], BF16, tag="e")
                zs = smalls.tile([128, 2], F32, tag="zs")
                for half in range(2):  # s in halves of 1024
                    pe_ = psum_e.tile([128, 1024], F32, tag="pe")
                    for g in range(2):
                        nc.tensor.matmul(
                            pe_[:, g * 512:(g + 1) * 512],
                            lhsT=w2s[pb:pb + 64, h, :],
                            rhs=hts[kc][pb:pb + 64,
                                        half * 1024 + g * 512:
                                        half * 1024 + (g + 1) * 512],
                            start=True,
                            stop=True,
                        )
                    nc.scalar.activation(
                        out=eh[:, half * 1024:(half + 1) * 1024],
                        in_=pe_,
                        func=AF.Exp,
                        accum_out=zs[:, half:half + 1],
                    )
                z = smalls.tile([128, 1], F32, tag="z")
                nc.vector.tensor_add(z, zs[:, 0:1], zs[:, 1:2])
                num = smalls.tile([128, 1], F32, tag="num")
                prod = prodpool.tile([128, S], BF16, tag="prod")
                # out = (e * 1.0) * fT ; accum_out = sum_s(out)   (TTR opcode
                # crashes the runtime here, scalar_tensor_tensor works)
                nc.vector.scalar_tensor_tensor(
                    out=prod,
                    in0=eh,
                    in1=ftt[h],
                    scalar=1.0,
                    op0=ALU.mult,
                    op1=ALU.mult,
                    accum_out=num,
                )
                rz = smalls.tile([128, 1], F32, tag="rz")
                nc.vector.reciprocal(rz, z)
                nc.vector.tensor_mul(outacc[:, b * H + h:b * H + h + 1], num, rz)

        nc.sync.dma_start(out=outp.ap(), in_=outacc)

    nc.compile()
    return nc


def prep_inputs(features, w1, b1, w2):
    """Host-side sharding/layout. Returns in_maps for 8 cores."""
    bf = ml_dtypes.bfloat16
    # W1[d, h*64+k] as lhsT chunks: w1t[p, dc, k'] = W1[dc*128+p, k']
    W1 = np.ascontiguousarray(w1.transpose(1, 0, 2).reshape(D, KP))
    w1t = np.ascontiguousarray(
        W1.reshape(8, 128, KP).transpose(1, 0, 2)).astype(bf)
    b1v = np.ascontiguousarray(
        b1.reshape(KP).reshape(4, 128).T).astype(np.float32)
    w2p = np.zeros((128, H, DHO), dtype=bf)
    for h in range(H):
        pb = (h % 2) * 64
        w2p[pb:pb + 64, h, :] = w2[h].astype(bf)

    in_maps = []
    for c in range(NCORES):
        fc = features[c * BL:(c + 1) * BL]  # [BL, S, D]
        ft = np.ascontiguousarray(fc.transpose(0, 2, 1)).astype(bf)
        ftp = ft.reshape(BL, 8, 128, S)
        in_maps.append({"ftp": ftp, "w1t": w1t, "b1v": b1v, "w2p": w2p})
    return in_maps


def assemble_output(results):
    """results: list of 8 dicts with 'outp' [128, BL*H] f32 -> [B, D]."""
    out = np.empty((B, D), dtype=np.float32)
    for c, r in enumerate(results):
        o = np.asarray(r["outp"], dtype=np.float32)  # [128(o), BL*H]
        blk = o.reshape(128, BL, H).transpose(1, 2, 0).reshape(BL, D)
        out[c * BL:(c + 1) * BL] = blk
    return out


_NC_CACHE = {}


def get_nc():
    if "nc" not in _NC_CACHE:
        _NC_CACHE["nc"] = build_bass()
    return _NC_CACHE["nc"]


def kernel(features, mask, lengths, w1, b1, w2, b2, **_ignored):
    # mask is all-ones and lengths unused in the reference forward; b2 is
    # constant along the softmax axis so it cancels in the softmax.
    features = np.asarray(features, dtype=np.float32)
    in_maps = prep_inputs(features, np.asarray(w1, np.float32),
                          np.asarray(b1, np.float32), np.asarray(w2, np.float32))
    nc = get_nc()
    res = run_bass_kernel_spmd(nc, in_maps, core_ids=list(range(NCORES)))
    return assemble_output(res.results)


if __name__ == "__main__":
    rng = np.random.default_rng(0)
    feats = rng.standard_normal((B, S, D), dtype=np.float32)
    w1 = (rng.standard_normal((H, D, DH)) * 0.01).astype(np.float32)
    b1 = (rng.standard_normal((H, DH)) * 0.01).astype(np.float32)
    w2 = (rng.standard_normal((H, DH, DHO)) * 0.01).astype(np.float32)
    b2 = (rng.standard_normal((H, DHO)) * 0.01).astype(np.float32)
    out = kernel(feats, np.ones((B, S), np.int32), None, w1, b1, w2, b2)
    print(out.shape, out.dtype, np.abs(out).mean())
